# revision 6
# baseline (speedup 1.0000x reference)
"""Trainium2 Bass kernel for nn_CATLayer (MoE expert-FFN + tiny expert-axis MHA).

Strategy:
- Data-parallel over B*S positions: 8192 positions -> 1024 per core, 8 cores.
- Only slice expert_id of the attention output survives, so Q / out-proj are
  computed for that single expert row; K/V for all experts.
- Feature-on-partition layout (xT/hT/eoT/kT/vT/qT all [feature, positions]):
  every matmul is lhsT(=host-pre-transposed weight tile) x moving positions.
- bf16 matmuls (fp32 PSUM accumulate), online-softmax accumulation over the
  expert axis (expert_id processed first so qT exists), fp32 o_run/sumexp.
- All transposes/tiling of weights happen host-side in kernel() (layout prep).
"""
import numpy as np
import ml_dtypes

import concourse.bass as bass
import concourse.mybir as mybir
import concourse.tile as tile
from concourse import bacc

BF16 = mybir.dt.bfloat16
F32 = mybir.dt.float32
AF = mybir.ActivationFunctionType
OP = mybir.AluOpType

# Problem shape (hardcoded per contract)
B, S, NE, E, H = 4, 2048, 8, 1024, 16
DH = E // H
N_CORES = 8


def emit(nc, tc, cfg, aps):
    """Emit the per-core kernel IR."""
    T, W, ET, FT, ne, eid = cfg["T"], cfg["W"], cfg["ET"], cfg["FT"], cfg["NE"], cfg["eid"]
    C = T // W
    scale = 1.0 / float(np.sqrt(DH))

    xT, w1t, w2t, wkt, wvt, wqt, owt = (
        aps["xT"], aps["w1t"], aps["w2t"], aps["wkt"], aps["wvt"], aps["wqt"], aps["owt"])
    b1t, b2t, bkt, bvt, bqt, obt = (
        aps["b1t"], aps["b2t"], aps["bkt"], aps["bvt"], aps["bqt"], aps["obt"])
    sel, bsel, out_d = aps["sel"], aps["bsel"], aps["out"]

    import contextlib
    ctx = contextlib.ExitStack()
    with ctx:
        persist = ctx.enter_context(tc.tile_pool(name="persist", bufs=1))
        wpool = ctx.enter_context(tc.tile_pool(name="wpool", bufs=12))
        kvwpool = ctx.enter_context(tc.tile_pool(name="kvwpool", bufs=12))
        evac = ctx.enter_context(tc.tile_pool(name="evac", bufs=2))
        attn = ctx.enter_context(tc.tile_pool(name="attn", bufs=3))
        biasp = ctx.enter_context(tc.tile_pool(name="biasp", bufs=2))
        psmm = ctx.enter_context(tc.tile_pool(name="psmm", bufs=4, space="PSUM"))
        pslg = ctx.enter_context(tc.tile_pool(name="pslg", bufs=2, space="PSUM"))
        psbc = ctx.enter_context(tc.tile_pool(name="psbc", bufs=2, space="PSUM"))

        # ---- persistent tiles ----
        xt_sb = [persist.tile([128, T], BF16, tag=f"xt{j}", name=f"xt{j}") for j in range(ET)]
        qt_sb = [persist.tile([128, T], BF16, tag=f"qt{j}", name=f"qt{j}") for j in range(ET)]
        ht_sb = [persist.tile([128, T], BF16, tag=f"ht{j}", name=f"ht{j}") for j in range(FT)]
        eot_sb = [persist.tile([128, T], BF16, tag=f"eot{j}", name=f"eot{j}") for j in range(ET)]
        kt_sb = [persist.tile([128, T], BF16, tag=f"kt{j}", name=f"kt{j}") for j in range(ET)]
        vt_sb = [persist.tile([128, T], BF16, tag=f"vt{j}", name=f"vt{j}") for j in range(ET)]
        orun_sb = [persist.tile([128, T], BF16, tag=f"orun{j}", name=f"orun{j}") for j in range(ET)]
        sume_sb = persist.tile([16, T], F32, tag="sume", name="sume")
        rec_sb = persist.tile([16, T], F32, tag="rec", name="rec")
        recb_sb = persist.tile([16, T], BF16, tag="recb", name="recb")
        sel_sb = [persist.tile([128, 16], BF16, tag=f"sel{j}", name=f"sel{j}") for j in range(ET)]
        bsel_sb = [persist.tile([16, 128], BF16, tag=f"bsel{j}", name=f"bsel{j}") for j in range(ET)]
        bk_sb = persist.tile([128, ET], F32, tag="bksb", name="bksb")
        bv_sb = persist.tile([128, ET], F32, tag="bvsb", name="bvsb")
        bq_sb = persist.tile([128, ET], F32, tag="bqsb", name="bqsb")
        ob_sb = persist.tile([128, ET], F32, tag="obsb", name="obsb")

        for j in range(ET):
            nc.sync.dma_start(xt_sb[j][:], xT[j])
            nc.sync.dma_start(sel_sb[j][:], sel[j])
            nc.sync.dma_start(bsel_sb[j][:], bsel[j])
        nc.sync.dma_start(bk_sb[:], bkt[:])
        nc.sync.dma_start(bv_sb[:], bvt[:])
        nc.sync.dma_start(bq_sb[:], bqt[:])
        nc.sync.dma_start(ob_sb[:], obt[:])

        order = [eid] + [n for n in range(ne) if n != eid]
        for n in order:
            b1_sb = biasp.tile([128, FT], F32, tag="b1sb", name=f"b1sb{n}")
            b2_sb = biasp.tile([128, ET], F32, tag="b2sb", name=f"b2sb{n}")
            nc.sync.dma_start(b1_sb[:], b1t[n])
            nc.sync.dma_start(b2_sb[:], b2t[n])

            # ---- FFN layer 1: hT[f, t] = relu(w1[n].T.T @ xT + b1) ----
            for ft in range(FT):
                ps = [psmm.tile([128, W], F32, tag="psmm", name=f"psh{n}_{ft}_{c}")
                      for c in range(C)]
                for et in range(ET):
                    w1_sb = wpool.tile([128, 128], BF16, tag="w1sb", name=f"w1sb{n}_{ft}_{et}")
                    nc.sync.dma_start(w1_sb[:], w1t[n, et, ft])
                    for c in range(C):
                        nc.tensor.matmul(ps[c][:], w1_sb[:], xt_sb[et][:, c * W:(c + 1) * W],
                                         start=(et == 0), stop=(et == ET - 1))
                for c in range(C):
                    nc.scalar.activation(ht_sb[ft][:, c * W:(c + 1) * W], ps[c][:],
                                         AF.Relu, bias=b1_sb[:, ft:ft + 1])

            # ---- FFN layer 2: eoT[e', t] = w2[n].T.T @ hT + b2 ----
            for et2 in range(ET):
                ps = [psmm.tile([128, W], F32, tag="psmm", name=f"pse{n}_{et2}_{c}")
                      for c in range(C)]
                for ft in range(FT):
                    w2_sb = wpool.tile([128, 128], BF16, tag="w2sb", name=f"w2sb{n}_{et2}_{ft}")
                    nc.sync.dma_start(w2_sb[:], w2t[n, ft, et2])
                    for c in range(C):
                        nc.tensor.matmul(ps[c][:], w2_sb[:], ht_sb[ft][:, c * W:(c + 1) * W],
                                         start=(ft == 0), stop=(ft == FT - 1))
                for c in range(C):
                    nc.scalar.activation(eot_sb[et2][:, c * W:(c + 1) * W], ps[c][:],
                                         AF.Identity, bias=b2_sb[:, et2:et2 + 1])

            # ---- K/V (+Q for eid) projections ----
            projs = [(wkt, kt_sb, bk_sb), (wvt, vt_sb, bv_sb)]
            if n == eid:
                projs.append((wqt, qt_sb, bq_sb))
            for w_d, dst, bias in projs:
                for et2 in range(ET):
                    ps = [psmm.tile([128, W], F32, tag="psmm", name=f"psp{n}_{et2}_{c}")
                          for c in range(C)]
                    for ei in range(ET):
                        wp_sb = kvwpool.tile([128, 128], BF16, tag="wpsb",
                                             name=f"wpsb{n}_{et2}_{ei}")
                        nc.sync.dma_start(wp_sb[:], w_d[ei, et2])
                        for c in range(C):
                            nc.tensor.matmul(ps[c][:], wp_sb[:],
                                             eot_sb[ei][:, c * W:(c + 1) * W],
                                             start=(ei == 0), stop=(ei == ET - 1))
                    for c in range(C):
                        nc.scalar.activation(dst[et2][:, c * W:(c + 1) * W], ps[c][:],
                                             AF.Identity, bias=bias[:, et2:et2 + 1])

            # ---- attention (online over experts) ----
            mask_val = 1.0 if n <= eid else 0.0
            for c in range(C):
                pl = pslg.tile([16, W], F32, tag="pslg", name=f"pl{n}_{c}")
                for j in range(ET):
                    prod = attn.tile([128, W], BF16, tag="prod", bufs=3,
                                     name=f"prod{n}_{c}_{j}")
                    nc.vector.tensor_tensor(prod[:], qt_sb[j][:, c * W:(c + 1) * W],
                                            kt_sb[j][:, c * W:(c + 1) * W], OP.mult)
                    nc.tensor.matmul(pl[:], sel_sb[j][:], prod[:],
                                     start=(j == 0), stop=(j == ET - 1))
                pexp = attn.tile([16, W], BF16, tag="pexp", bufs=3, name=f"pexp{n}_{c}")
                nc.scalar.activation(pexp[:], pl[:], AF.Exp, bias=mask_val, scale=scale)
                if n == order[0]:
                    nc.vector.tensor_copy(sume_sb[:, c * W:(c + 1) * W], pexp[:])
                else:
                    nc.vector.tensor_tensor(sume_sb[:, c * W:(c + 1) * W],
                                            sume_sb[:, c * W:(c + 1) * W], pexp[:], OP.add)
                for j in range(ET):
                    pb = psbc.tile([128, W], F32, tag="psbc", name=f"pb{n}_{c}_{j}")
                    nc.tensor.matmul(pb[:], bsel_sb[j][:], pexp[:], start=True, stop=True)
                    avp = attn.tile([128, W], BF16, tag="avp", bufs=3, name=f"avp{n}_{c}_{j}")
                    nc.vector.tensor_tensor(avp[:], pb[:], vt_sb[j][:, c * W:(c + 1) * W],
                                            OP.mult)
                    if n == order[0]:
                        nc.vector.tensor_copy(orun_sb[j][:, c * W:(c + 1) * W], avp[:])
                    else:
                        nc.vector.tensor_tensor(orun_sb[j][:, c * W:(c + 1) * W],
                                                orun_sb[j][:, c * W:(c + 1) * W],
                                                avp[:], OP.add)

        # ---- normalize: oT = o_run * (1/sumexp) broadcast ----
        ot_sb = eot_sb  # eoT tiles are dead after the last expert; reuse for oT
        nc.vector.reciprocal(rec_sb[:], sume_sb[:])
        nc.scalar.copy(recb_sb[:], rec_sb[:])
        for c in range(C):
            for j in range(ET):
                pb = psbc.tile([128, W], F32, tag="psbc", name=f"pbr{c}_{j}")
                nc.tensor.matmul(pb[:], bsel_sb[j][:], recb_sb[:, c * W:(c + 1) * W],
                                 start=True, stop=True)
                nc.vector.tensor_tensor(ot_sb[j][:, c * W:(c + 1) * W],
                                        orun_sb[j][:, c * W:(c + 1) * W], pb[:], OP.mult)

        # ---- output projection: outT[eo, t] = out_w.T.T @ oT + out_b ----
        for eo in range(ET):
            ps = [psmm.tile([128, W], F32, tag="psmm", name=f"pso{eo}_{c}")
                  for c in range(C)]
            for ei in range(ET):
                ow_sb = kvwpool.tile([128, 128], BF16, tag="owsb", name=f"owsb{eo}_{ei}")
                nc.sync.dma_start(ow_sb[:], owt[ei, eo])
                for c in range(C):
                    nc.tensor.matmul(ps[c][:], ow_sb[:], ot_sb[ei][:, c * W:(c + 1) * W],
                                     start=(ei == 0), stop=(ei == ET - 1))
            for c in range(C):
                outt = evac.tile([128, W], F32, tag="outt", name=f"outt{eo}_{c}")
                nc.scalar.activation(outt[:], ps[c][:],
                                     AF.Identity, bias=ob_sb[:, eo:eo + 1])
                nc.sync.dma_start(out_d[eo][:, c * W:(c + 1) * W], outt[:])


def build(cfg):
    T, ET, FT, ne = cfg["T"], cfg["ET"], cfg["FT"], cfg["NE"]
    nc = bacc.Bacc("TRN2", target_bir_lowering=False, debug=False,
                   num_devices=cfg.get("num_devices", N_CORES))
    aps = {}
    def din(name, shape, dt):
        aps[name] = nc.dram_tensor(name, shape, dt, kind="ExternalInput").ap()
    din("xT", [ET, 128, T], BF16)
    din("w1t", [ne, ET, FT, 128, 128], BF16)
    din("w2t", [ne, FT, ET, 128, 128], BF16)
    din("wkt", [ET, ET, 128, 128], BF16)
    din("wvt", [ET, ET, 128, 128], BF16)
    din("wqt", [ET, ET, 128, 128], BF16)
    din("owt", [ET, ET, 128, 128], BF16)
    din("b1t", [ne, 128, FT], F32)
    din("b2t", [ne, 128, ET], F32)
    din("bkt", [128, ET], F32)
    din("bvt", [128, ET], F32)
    din("bqt", [128, ET], F32)
    din("obt", [128, ET], F32)
    din("sel", [ET, 128, 16], BF16)
    din("bsel", [ET, 16, 128], BF16)
    aps["out"] = nc.dram_tensor("out", [ET, 128, T], F32, kind="ExternalOutput").ap()
    with tile.TileContext(nc) as tc:
        emit(nc, tc, cfg, aps)
    nc.compile()
    return nc


# ---------------- host side ----------------

def _tile2(w):
    """[O, I] weight -> lhsT tiles [I/128, O/128, 128, 128] (K=i, M=o)."""
    O, I = w.shape
    return np.ascontiguousarray(
        w.reshape(O // 128, 128, I // 128, 128).transpose(2, 0, 3, 1))


def _bf16(x):
    return np.asarray(x, np.float32).astype(ml_dtypes.bfloat16)


def prep_inputs(x, w1, b1, w2, b2, in_proj_w, in_proj_b, out_w, out_b, cfg):
    """Returns (common dict, per-core xT list)."""
    ET, FT, ne, T = cfg["ET"], cfg["FT"], cfg["NE"], cfg["T"]
    Ee = ET * 128
    Ff = FT * 128
    w1 = np.asarray(w1, np.float32); w2 = np.asarray(w2, np.float32)
    ipw = np.asarray(in_proj_w, np.float32); ipb = np.asarray(in_proj_b, np.float32)
    out_w = np.asarray(out_w, np.float32)
    b1 = np.asarray(b1, np.float32); b2 = np.asarray(b2, np.float32)
    out_b = np.asarray(out_b, np.float32)

    com = {}
    com["w1t"] = _bf16(np.stack([_tile2(w1[n]) for n in range(ne)]))
    # w2 lhsT tiles: [K=f, M=e'] -> _tile2(w2[n]) gives [f/128, e'/128,...]
    com["w2t"] = _bf16(np.stack([_tile2(w2[n]) for n in range(ne)]))
    wq, wk, wv = ipw[:Ee], ipw[Ee:2 * Ee], ipw[2 * Ee:3 * Ee]
    com["wkt"] = _bf16(_tile2(wk))
    com["wvt"] = _bf16(_tile2(wv))
    com["wqt"] = _bf16(_tile2(wq))
    com["owt"] = _bf16(_tile2(out_w))
    com["b1t"] = np.ascontiguousarray(
        b1.reshape(ne, FT, 128).transpose(0, 2, 1)).astype(np.float32)
    com["b2t"] = np.ascontiguousarray(
        b2.reshape(ne, ET, 128).transpose(0, 2, 1)).astype(np.float32)
    bq, bk, bv = ipb[:Ee], ipb[Ee:2 * Ee], ipb[2 * Ee:3 * Ee]
    for nm, b in (("bkt", bk), ("bvt", bv), ("bqt", bq), ("obt", out_b)):
        com[nm] = np.ascontiguousarray(b.reshape(ET, 128).T).astype(np.float32)
    # selectors
    p = np.arange(128)
    sel = np.zeros((ET, 128, 16), np.float32)
    bsl = np.zeros((ET, 16, 128), np.float32)
    for j in range(ET):
        heads = 2 * j + p // 64           # head index of each partition
        sel[j, p, heads] = 1.0
        bsl[j, heads, p] = 1.0
    com["sel"] = _bf16(sel)
    com["bsel"] = _bf16(bsl)
    return com


_CACHE = {}


def _get_runner(cfg_key, cfg):
    if cfg_key not in _CACHE:
        nc = build(cfg)
        from kernel_runner import BassRunner
        _CACHE[cfg_key] = BassRunner(nc, cfg.get("num_devices", N_CORES))
    return _CACHE[cfg_key]


def kernel(x, w1, b1, w2, b2, in_proj_w, in_proj_b, out_w, out_b, expert_id):
    eid = int(expert_id)
    cfg = {"T": (B * S) // N_CORES, "W": 512, "ET": E // 128, "FT": (4 * E) // 128,
           "NE": NE, "eid": eid, "num_devices": N_CORES}
    runner = _get_runner(("main", eid), cfg)
    com = prep_inputs(x, w1, b1, w2, b2, in_proj_w, in_proj_b, out_w, out_b, cfg)
    x2 = np.asarray(x, np.float32).reshape(B * S, E)
    T = cfg["T"]
    in_maps = []
    for c in range(N_CORES):
        xs = _bf16(x2[c * T:(c + 1) * T].T.reshape(E // 128, 128, T))
        m = dict(com)
        m["xT"] = xs
        in_maps.append(m)
    res = runner(in_maps)
    outs = []
    for c in range(N_CORES):
        o = res[c]["out"]          # [ET, 128, T]
        outs.append(o.transpose(2, 0, 1).reshape(T, E))
    full = np.concatenate(outs, axis=0).reshape(B, S, E)
    return full.astype(np.float32)


# -------- embedded runner (self-contained; no sibling imports) --------
import sys as _sys
import types as _types

_RUNNER_SRC = '''
import numpy as np
import jax
from jax.sharding import Mesh, PartitionSpec
from jax.experimental.shard_map import shard_map
import concourse.mybir as mybir
from concourse.bass2jax import _bass_exec_p, install_neuronx_cc_hook, partition_id_tensor


class BassRunner:
    def __init__(self, nc, n_cores):
        install_neuronx_cc_hook()
        self.nc = nc
        self.n_cores = n_cores
        partition_name = nc.partition_id_tensor.name if nc.partition_id_tensor else None
        in_names, out_names, out_avals, zero_outs = [], [], [], []
        for alloc in nc.m.functions[0].allocations:
            if not isinstance(alloc, mybir.MemoryLocationSet):
                continue
            name = alloc.memorylocations[0].name
            if alloc.kind == "ExternalInput":
                if name != partition_name:
                    in_names.append(name)
            elif alloc.kind == "ExternalOutput":
                shape = tuple(alloc.tensor_shape)
                dtype = mybir.dt.np(alloc.dtype)
                out_names.append(name)
                out_avals.append(jax.core.ShapedArray(shape, dtype))
                zero_outs.append(np.zeros(shape, dtype))
        self.in_names, self.out_names = in_names, out_names
        self.zero_outs = zero_outs
        n_params = len(in_names)
        n_outs = len(out_avals)
        all_in_names = list(in_names) + list(out_names)
        if partition_name is not None:
            all_in_names.append(partition_name)
        donate = tuple(range(n_params, n_params + n_outs))

        def _body(*args):
            operands = list(args)
            if partition_name is not None:
                operands.append(partition_id_tensor())
            outs = _bass_exec_p.bind(
                *operands,
                out_avals=tuple(out_avals),
                in_names=tuple(all_in_names),
                out_names=tuple(out_names),
                lowering_input_output_aliases=(),
                sim_require_finite=True,
                sim_require_nnan=True,
                nc=nc,
            )
            return tuple(outs)

        if n_cores == 1:
            self.fn = jax.jit(_body, donate_argnums=donate, keep_unused=True)
        else:
            devices = jax.devices()[:n_cores]
            mesh = Mesh(np.asarray(devices), ("core",))
            in_specs = (PartitionSpec("core"),) * (n_params + n_outs)
            out_specs = (PartitionSpec("core"),) * n_outs
            self.fn = jax.jit(
                shard_map(_body, mesh=mesh, in_specs=in_specs, out_specs=out_specs,
                          check_rep=False),
                donate_argnums=donate, keep_unused=True)

    def __call__(self, in_maps):
        n = self.n_cores
        if n == 1:
            args = [np.asarray(in_maps[0][k]) for k in self.in_names]
            outs = self.fn(*args, *[np.zeros_like(z) for z in self.zero_outs])
            return [{k: np.asarray(outs[i]) for i, k in enumerate(self.out_names)}]
        concat_in = [np.concatenate([np.asarray(in_maps[c][k]) for c in range(n)], axis=0)
                     for k in self.in_names]
        concat_zero = [np.zeros((n * z.shape[0], *z.shape[1:]), z.dtype)
                       for z in self.zero_outs]
        outs = self.fn(*concat_in, *concat_zero)
        res = []
        for c in range(n):
            d = {}
            for i, k in enumerate(self.out_names):
                arr = outs[i]
                per = arr.shape[0] // n
                d[k] = np.asarray(arr[c * per:(c + 1) * per])
            res.append(d)
        return res
'''

if "kernel_runner" not in _sys.modules:
    _mod = _types.ModuleType("kernel_runner")
    exec(_RUNNER_SRC, _mod.__dict__)
    _sys.modules["kernel_runner"] = _mod
